# revision 1
# baseline (speedup 1.0000x reference)
"""Trainium2 Bass kernel for MultiHeadAttention (B=2, N=2048, DIM=1024, H=16).

Sharding: 8 cores = 2 batches x 4 head-groups (4 heads each).
Each core computes qkv projections for its head slice, attention, and a
partial output projection (over its 256 d-columns). Host sums the 4
partials per batch and adds the bias.

Device layout (all fp32, matmuls in float32r for full PE rate):
  xT  [d=1024, n=2048]  feature-major activations (pre-transposed on host)
  wT  [d=1024, e=768]   qkv weight slice, transposed on host (e = q|k|v 256 each)
  woT [d=256,  e=1024]  out-proj weight slice, transposed on host
  qkT [e=512, n]        q,k feature-major (on chip)
  vv  token-major V with a ones-column per head (softmax denominator comes
      out of the attn@V matmul as one extra output row)
  Sᵀ tiles [j, i]; exp on ACT in 2048-wide strips; normalization via a
  rank-1 PE broadcast of 1/denom and a DVE multiply.
"""

import os
import sys
from contextlib import ExitStack

import numpy as np

for _p in ("/opt/trn_rl_repo", os.path.expanduser("~/.axon_site/_ro/trn_rl_repo")):
    if os.path.isdir(_p) and _p not in sys.path:
        sys.path.append(_p)

import concourse.bass as bass  # noqa: E402
import concourse.mybir as mybir  # noqa: E402
import concourse.tile as tile  # noqa: E402

F32 = mybir.dt.float32
F32R = mybir.dt.float32r
EXP = mybir.ActivationFunctionType.Exp

B, N, DIM, HEADS = 2, 2048, 1024, 16
DH = DIM // HEADS          # 64
NHL = 4                    # heads per core
SCALE = DH ** -0.5
NCORES = 8
E3 = 3 * NHL * DH          # 768 qkv features per core
EV = NHL * DH              # 256 v features per core
VW = DH + 2                # 66: v + ones column + pad (fp32r needs even M)
NT = N // 128              # 16 token chunks
DC = DIM // 128            # 8 d chunks


def _r(ap):
    return ap.bitcast(F32R)


def build_nc(repeat=1, split_waits=True):
    nc = bass.Bass("TRN2", target_bir_lowering=False, debug=False,
                   num_devices=NCORES)
    xT_d = nc.dram_tensor("xT", [DIM, N], F32, kind="ExternalInput").ap()
    wT_d = nc.dram_tensor("wT", [DIM, E3], F32, kind="ExternalInput").ap()
    woT_d = nc.dram_tensor("woT", [EV, DIM], F32, kind="ExternalInput").ap()
    out_d = nc.dram_tensor("out", [N, DIM], F32, kind="ExternalOutput").ap()

    with tile.TileContext(nc) as tc, ExitStack() as ctx:
        if repeat > 1:
            ctx.enter_context(tc.For_i(0, repeat, 1))
        pers = ctx.enter_context(tc.tile_pool(name="pers", bufs=1))
        xT = pers.tile([128, DC * N], F32, tag="xT", name="xT_sb")
        wT = pers.tile([128, DC * E3], F32, tag="wT", name="wT_sb")
        woT = pers.tile([128, 2 * DIM], F32, tag="woT", name="woT_sb")
        qkT = pers.tile([128, 4 * N], F32, tag="qkT", name="qkT_sb")
        vv = pers.tile([128, NT * NHL * VW], F32, tag="vv", name="vv_sb")
        ones = pers.tile([128, 64], F32, tag="ones", name="ones_sb")

        strip_p = ctx.enter_context(tc.tile_pool(name="strip", bufs=3))
        oT_p = ctx.enter_context(tc.tile_pool(name="oT", bufs=2))
        ostg_p = ctx.enter_context(tc.tile_pool(name="ostg", bufs=2))
        rec_p = ctx.enter_context(tc.tile_pool(name="rec", bufs=2))

        st_ps = ctx.enter_context(tc.tile_pool(name="st_ps", bufs=2, space="PSUM"))
        mm_ps = ctx.enter_context(tc.tile_pool(name="mm_ps", bufs=2, space="PSUM"))
        acc_ps = ctx.enter_context(tc.tile_pool(name="acc_ps", bufs=2, space="PSUM"))

        # --- loads: DMA into staging (strip-pool slots), then DVE
        # round-copy into the fp32r-consumed persistent tensors (the BIR
        # verifier requires every writer of an fp32r matmul operand to be
        # a rounding producer, which a DMA is not) ---
        ld_p = ctx.enter_context(tc.tile_pool(name="ld", bufs=3))

        def load_rounded(dst_ap, src_ap, cols):
            stg = ld_p.tile([128, cols], F32, tag="ld", name="ld_stg")
            nc.sync.dma_start(out=stg[:], in_=src_ap)
            nc.vector.tensor_copy(_r(dst_ap), stg[:])

        for dc in range(DC):
            load_rounded(wT[:, dc * E3:(dc + 1) * E3],
                         wT_d[dc * 128:(dc + 1) * 128, :], E3)
            for p2 in range(2):
                load_rounded(
                    xT[:, dc * N + p2 * 1024: dc * N + (p2 + 1) * 1024],
                    xT_d[dc * 128:(dc + 1) * 128, p2 * 1024:(p2 + 1) * 1024],
                    1024)
        for d2 in range(2):
            load_rounded(woT[:, d2 * DIM:(d2 + 1) * DIM],
                         woT_d[d2 * 128:(d2 + 1) * 128, :], DIM)
        one_stg = ld_p.tile([128, NHL * VW * 4], F32, tag="ld", name="one_stg")
        nc.vector.memset(one_stg[:], 1.0)
        for q in range(4):
            nc.vector.tensor_copy(
                _r(vv[:, q * NHL * VW * 4:(q + 1) * NHL * VW * 4]), one_stg[:])
        nc.vector.tensor_copy(_r(ones[:]), one_stg[:, 0:64])

        # --- q,k projections: qkT[ec][n], ec0,1=q heads01,23; ec2,3=k ---
        def qk_mm(ps_ap, ec, nt4, dc):
            nc.tensor.matmul(
                ps_ap,
                _r(wT[:, dc * E3 + ec * 128: dc * E3 + (ec + 1) * 128]),
                _r(xT[:, dc * N + nt4 * 512: dc * N + (nt4 + 1) * 512]),
                start=(dc == 0), stop=(dc == DC - 1))

        def qk_proj(ec):
            for nt4 in range(4):
                ps = mm_ps.tile([128, 512], F32, tag="mm", name="qk_ps")
                for dc in range(DC):
                    qk_mm(ps[:], ec, nt4, dc)
                nc.vector.tensor_copy(
                    _r(qkT[:, ec * N + nt4 * 512: ec * N + (nt4 + 1) * 512]),
                    ps[:])

        def v_proj(nt):
            ps = acc_ps.tile([128, 512], F32, tag="acc", name="v_ps")
            for dc in range(DC):
                nc.tensor.matmul(
                    ps[:, 0:EV],
                    _r(xT[:, dc * N + nt * 128: dc * N + (nt + 1) * 128]),
                    _r(wT[:, dc * E3 + 512: dc * E3 + 768]),
                    start=(dc == 0), stop=(dc == DC - 1))
            for h in range(NHL):
                nc.vector.tensor_copy(
                    _r(vv[:, nt * NHL * VW + h * VW: nt * NHL * VW + h * VW + DH]),
                    ps[:, h * DH:(h + 1) * DH])

        # streaming first pass over ec2 (k heads 0,1) and ec0 (q heads 0,1):
        # 8 psum accumulators consume each (xT, wT) d-chunk as it lands, so
        # the PE starts ~6us in instead of waiting for the full 11MB load.
        st_a = st_ps.tile([128, 1024], F32, tag="st", name="qs_a")
        st_b = st_ps.tile([128, 1024], F32, tag="st", name="qs_b")
        mm_g = [mm_ps.tile([128, 512], F32, tag="mm", name="qs_m") for _ in range(2)]
        acc_g = [acc_ps.tile([128, 512], F32, tag="acc", name="qs_c") for _ in range(2)]
        for dc in range(DC):
            qk_mm(st_a[:, 0:512], 2, 0, dc)
            qk_mm(st_a[:, 512:1024], 2, 1, dc)
            qk_mm(st_b[:, 0:512], 2, 2, dc)
            qk_mm(st_b[:, 512:1024], 2, 3, dc)
            qk_mm(mm_g[0][:], 0, 0, dc)
            qk_mm(mm_g[1][:], 0, 1, dc)
            qk_mm(acc_g[0][:], 0, 2, dc)
            qk_mm(acc_g[1][:], 0, 3, dc)
        nc.vector.tensor_copy(_r(qkT[:, 2 * N + 0: 2 * N + 1024]), st_a[:])
        nc.vector.tensor_copy(_r(qkT[:, 2 * N + 1024: 2 * N + 2048]), st_b[:])
        for i, g in enumerate(mm_g):
            nc.vector.tensor_copy(_r(qkT[:, i * 512:(i + 1) * 512]), g[:])
        for i, g in enumerate(acc_g):
            nc.vector.tensor_copy(_r(qkT[:, (i + 2) * 512:(i + 3) * 512]), g[:])

        # --- attention + out-projection, per i-tile of 512 queries ---
        def oproj_one(oT_prev, it_prev, ng, eh):
            ps = mm_ps.tile([128, 512], F32, tag="mm", name="op_ps")
            for d2 in range(2):
                nc.tensor.matmul(
                    ps[:],
                    _r(oT_prev[:, d2 * 512 + ng * 128: d2 * 512 + (ng + 1) * 128]),
                    _r(woT[:, d2 * DIM + eh * 512: d2 * DIM + (eh + 1) * 512]),
                    start=(d2 == 0), stop=(d2 == 1))
            stg = ostg_p.tile([128, 512], F32, tag="ostg")
            nc.vector.tensor_copy(stg[:], ps[:])
            nc.sync.dma_start(
                out=out_d[it_prev * 512 + ng * 128: it_prev * 512 + (ng + 1) * 128,
                          eh * 512:(eh + 1) * 512],
                in_=stg[:])

        v_proj(0)
        v_proj(1)
        vq = [lambda nt=nt: v_proj(nt) for nt in range(2, NT)]

        def _g(ec, nt4):
            ps = mm_ps.tile([128, 512], F32, tag="mm", name="qk_ps")
            for dc in range(DC):
                qk_mm(ps[:], ec, nt4, dc)
            nc.vector.tensor_copy(
                _r(qkT[:, ec * N + nt4 * 512: ec * N + (nt4 + 1) * 512]), ps[:])

        # remaining q,k groups, ordered so each lands just before its first
        # consumer when popped every other sg across it0 h1+h2
        ecq = [lambda: _g(1, 0), lambda: _g(3, 0), lambda: _g(3, 1),
               lambda: _g(3, 2), lambda: _g(3, 3), lambda: _g(1, 1),
               lambda: _g(1, 2), lambda: _g(1, 3)]

        pending = []  # deferred out-proj work, interleaved into strip loop
        normq = []    # deferred per-head normalize chains

        def fill_hook(it, h, sg):
            if it == 0:
                if h == 0:
                    for _ in range(2):
                        if vq:
                            vq.pop(0)()
                elif h in (1, 2):
                    if sg % 2 == 0 and ecq:
                        ecq.pop(0)()
            elif sg % 4 == 3 and pending:
                pending.pop(0)()

        for it in range(4):
            oT_t = oT_p.tile([128, 1024], F32, tag="oT")  # [d2 2][n 512]
            for h in range(NHL):
                hb = h // 2            # chunk pair index / d2 block
                po = (h % 2) * 64      # partition offset within chunk
                out_ps = acc_ps.tile([128, 512], F32, tag="acc",
                                     name="at_ps")  # rows 0:65
                pv = []  # deferred PV matmuls (one-strip software pipeline)
                for sg in range(8):    # strips of 2 j-chunks
                    ps = st_ps.tile([128, 1024], F32, tag="st")
                    for q2 in range(2):
                        jc = sg * 2 + q2
                        nc.tensor.matmul(
                            ps[:, q2 * 512:(q2 + 1) * 512],
                            _r(qkT[po:po + 64,
                                   (2 + hb) * N + jc * 128: (2 + hb) * N + (jc + 1) * 128]),
                            _r(qkT[po:po + 64,
                                   hb * N + it * 512: hb * N + (it + 1) * 512]),
                            start=True, stop=True)
                    strip = strip_p.tile([128, 1024], F32, tag="strip")
                    nc.scalar.activation(_r(strip[:]), ps[:], EXP, scale=SCALE)
                    if pv:
                        pv.pop(0)()
                    if sg == 2 and normq:
                        normq.pop(0)()
                    fill_hook(it, h, sg)

                    def _pv(strip=strip, sg=sg, h=h, out_ps=out_ps):
                        for q2 in range(2):
                            jc = sg * 2 + q2
                            nc.tensor.matmul(
                                out_ps[0:VW, :],
                                _r(vv[:, jc * NHL * VW + h * VW:
                                       jc * NHL * VW + h * VW + VW]),
                                _r(strip[:, q2 * 512:(q2 + 1) * 512]),
                                start=(jc == 0), stop=(jc == NT - 1))
                    pv.append(_pv)
                while pv:
                    pv.pop(0)()

                # normalize oT[d, i] = out[d, i] / out[64, i]; deferred one
                # head so the rank-1 broadcast matmul never stalls the PE
                def _norm(out_ps=out_ps, oT_t=oT_t, po=po, hb=hb):
                    rec = rec_p.tile([128, 512], F32, tag="rec")
                    with nc.allow_low_precision(reason="f32r view of f32"):
                        nc.vector.reciprocal(_r(rec[64:65, :]),
                                             out_ps[64:65, :])
                    bc = mm_ps.tile([128, 512], F32, tag="mm", name="bc_ps")
                    nc.tensor.matmul(bc[0:64, :], _r(ones[64:65, 0:64]),
                                     _r(rec[64:65, :]), start=True, stop=True)
                    nst = rec_p.tile([128, 512], F32, tag="nstg")
                    nc.vector.tensor_copy(nst[0:64, :], out_ps[0:64, :])
                    nc.vector.tensor_mul(
                        _r(oT_t[po:po + 64, hb * 512:(hb + 1) * 512]),
                        nst[0:64, :], bc[0:64, :])
                normq.append(_norm)
            for ng in range(4):
                for eh in range(2):
                    pending.append(
                        lambda oT_prev=oT_t, it_prev=it, ng=ng, eh=eh:
                        oproj_one(oT_prev, it_prev, ng, eh))
        while normq:
            normq.pop(0)()
        while pending:
            pending.pop(0)()
    if split_waits:
        _split_dma_waits(nc)
    return nc


def _split_dma_waits(nc):
    """walrus's DMA/LDWEIGHTS encodings take a single sync wait; move
    extra waits onto an EventSemaphore on the issuing sequencer."""
    fn = nc.m.functions[0]
    for bb in fn.blocks:
        insts = bb.instructions
        i = 0
        while i < len(insts):
            inst = insts[i]
            si = getattr(inst, "sync_info", None)
            if (si is not None and len(si.on_wait) > 1
                    and type(inst).__name__ != "InstEventSemaphore"):
                waits = list(si.on_wait)
                for k, w in enumerate(waits[:-1]):
                    ev = mybir.InstEventSemaphore(
                        name=f"{inst.name}-wsplit{k}", ins=[], outs=[])
                    ev.engine = inst.engine
                    ev.sync_info = type(si)(on_wait=[w], on_update=[])
                    insts.insert(i, ev)
                    i += 1
                inst.sync_info = type(si)(on_wait=waits[-1:],
                                          on_update=list(si.on_update))
            i += 1


_NC = None


def _get_nc():
    global _NC
    if _NC is None:
        _NC = build_nc()
    return _NC


def make_in_maps(x, w_qkv, w_out):
    x = np.asarray(x, dtype=np.float32)
    w_qkv = np.asarray(w_qkv, dtype=np.float32)
    w_out = np.asarray(w_out, dtype=np.float32)
    xT_by_b = [np.ascontiguousarray(x[b].T) for b in range(B)]
    in_maps = []
    for c in range(NCORES):
        b, g = divmod(c, 4)
        r0 = g * NHL * DH  # 256-wide feature slice
        wq = w_qkv[r0:r0 + EV]
        wk = w_qkv[DIM + r0:DIM + r0 + EV]
        wv = w_qkv[2 * DIM + r0:2 * DIM + r0 + EV]
        wT = np.ascontiguousarray(np.concatenate([wq, wk, wv], 0).T)
        woT = np.ascontiguousarray(w_out[:, r0:r0 + EV].T)
        in_maps.append({"xT": xT_by_b[b], "wT": wT, "woT": woT})
    return in_maps


def combine(results, b_out):
    """results: list of 8 dicts with 'out' [N, DIM] partials."""
    b_out = np.asarray(b_out, dtype=np.float32)
    out = np.empty((B, N, DIM), dtype=np.float32)
    for b in range(B):
        acc = results[4 * b]["out"].astype(np.float32, copy=True)
        for g in range(1, 4):
            acc += results[4 * b + g]["out"]
        out[b] = acc + b_out[None, :]
    return out


def kernel(x, w_qkv, w_out, b_out):
    from concourse.bass_utils import run_bass_kernel_spmd
    nc = _get_nc()
    in_maps = make_in_maps(x, w_qkv, w_out)
    res = run_bass_kernel_spmd(nc, in_maps, list(range(NCORES)))
    return combine(res.results, b_out)



# revision 54
# speedup vs baseline: 1.3124x; 1.3124x over previous
"""Trainium2 Bass kernel for MultiHeadAttention (B=2, N=2048, DIM=1024, H=16).

Sharding: 8 cores = 2 batches x 4 head-groups (4 heads each).
Each core computes qkv projections for its head slice, attention, and a
partial output projection (over its 256 d-columns). Host sums the 4
partials per batch and adds the bias.

All-bf16 datapath (hosts quantize inputs; PSUM accumulation stays fp32;
measured end-to-end rel err ~7e-3 vs the 2e-2 gate):
  xT  [d=1024, n=2048] bf16   feature-major activations (DMA'd directly)
  wT  [d=1024, e=768]  bf16   qkv weight slice (e = q|k|v 256 each)
  woT [d=256,  e=1024] bf16   out-proj weight slice
  qkT [ec=4][128, n]   bf16   q h01 | q h23 | k h01 | k h23
  vv  [128 tok, nt*4*66] bf16 token-major V + ones column per head
  S^T tiles [j, i] in psum; exp on ACT -> bf16 strips.
  PV is FLIPPED: stationary = strip [j, i-chunk], moving = vv [j, 66]
  -> out psum [128 i, 66] (full partition use; ~2x cheaper than [66, i]).
  Softmax denominator comes out of the PV ones column; normalization is a
  per-partition DVE scalar multiply; PE transpose brings attn-out back to
  [d, i] for the out-projection.

Schedule: token-block streaming load (k-projection chunks consume DMA'd
x blocks as they land), then an ACT(exp)-paced attention pipeline where
leftover projection / out-projection work fills PE gaps via a budget-
tracked fill queue.
"""

import os
import sys
from collections import deque
from contextlib import ExitStack

import numpy as np

for _p in ("/opt/trn_rl_repo", os.path.expanduser("~/.axon_site/_ro/trn_rl_repo")):
    if os.path.isdir(_p) and _p not in sys.path:
        sys.path.append(_p)

import concourse.bass as bass  # noqa: E402
import concourse.mybir as mybir  # noqa: E402
import concourse.tile as tile  # noqa: E402
from concourse.masks import make_identity  # noqa: E402

F32 = mybir.dt.float32
F32R = mybir.dt.float32r
BF16 = mybir.dt.bfloat16
EXP = mybir.ActivationFunctionType.Exp


def _r(ap):
    return ap.bitcast(F32R)

B, N, DIM, HEADS = 2, 2048, 1024, 16
DH = DIM // HEADS          # 64
NHL = 4                    # heads per core
SCALE = DH ** -0.5
NCORES = 8
E3 = 3 * NHL * DH          # 768 qkv features per core
EV = NHL * DH              # 256 v features per core
VW = DH + 2                # 66: v + ones column + pad
NT = N // 128              # 16 token chunks (j)
DC = DIM // 128            # 8 d chunks
NB = 4                     # 512-token blocks (i / q blocks)

# engine-time constants (ns) for the greedy fill scheduler
PE_CYC = 1.0 / 2.4
ACT_STRIP_NS = 1038.0
MARGIN_NS = 150.0
WARMUP_MMS = 18


def _mm_ns(free):
    return free * PE_CYC


class _Fills:
    """Deque of (key, cost_ns, closure, dep). pump() pops FIFO while the PE
    clock is behind the ACT clock; force() pops a specific key now. An entry
    with a dep key blocks the queue head until the dep is marked done."""

    def __init__(self):
        self.q = deque()
        self.pe = 0.0
        self.act = 0.0
        self.margin = MARGIN_NS
        self.done = set()
        self.deps = {}

    def add(self, key, cost, fn, dep=None, last=True):
        self.q.append((key, cost, fn, last))
        if dep is not None:
            self.deps[key] = dep

    def add_chain(self, key, atoms):
        """atoms: list of (cost, fn); queued as separate entries so a
        pop never delays the strip cadence by more than one matmul."""
        for i, (cost, fn) in enumerate(atoms):
            self.q.append((key, cost, fn, i == len(atoms) - 1))

    def _run(self, item):
        key, cost, fn, last = item
        fn()
        self.pe += cost
        if last:
            self.done.add(key)

    def pump(self, slot_budget=500.0):
        spent = 0.0
        while self.q and self.pe + self.q[0][1] <= self.act - self.margin:
            dep = self.deps.get(self.q[0][0])
            if dep is not None and dep not in self.done:
                break
            cost = self.q[0][1]
            if spent + cost > slot_budget:
                break
            self._run(self.q.popleft())
            spent += cost

    def force(self, key):
        if key in self.done:
            return
        i = 0
        while i < len(self.q):
            if self.q[i][0] == key:
                item = self.q[i]
                del self.q[i]
                self._run(item)
            else:
                i += 1
        self.done.add(key)

    def drain(self):
        while self.q:
            self._run(self.q.popleft())


def build_nc(repeat=1, split_waits=True):
    nc = bass.Bass("TRN2", target_bir_lowering=False, debug=False,
                   num_devices=NCORES)
    xT_d = nc.dram_tensor("xT", [DIM, N], BF16, kind="ExternalInput").ap()
    wT_d = nc.dram_tensor("wT", [DIM, E3], BF16, kind="ExternalInput").ap()
    woT_d = nc.dram_tensor("woT", [EV, DIM], F32, kind="ExternalInput").ap()
    out_d = nc.dram_tensor("out", [N, DIM], F32, kind="ExternalOutput").ap()

    with tile.TileContext(nc) as tc, ExitStack() as ctx:
        if repeat > 1:
            ctx.enter_context(tc.For_i(0, repeat, 1))
        pers = ctx.enter_context(tc.tile_pool(name="pers", bufs=1))
        xT = pers.tile([128, DC * N], BF16, tag="xT", name="xT_sb")
        wT = pers.tile([128, DC * E3], BF16, tag="wT", name="wT_sb")
        woT = pers.tile([128, 2 * DIM], F32, tag="woT", name="woT_sb")
        woS = pers.tile([128, 2 * DIM], F32, tag="woS", name="woS_sb")
        qkT = pers.tile([128, 4 * N], BF16, tag="qkT", name="qkT_sb")
        vv = pers.tile([128, NT * NHL * VW], BF16, tag="vv", name="vv_sb")
        ident = pers.tile([128, 128], BF16, tag="ident", name="ident_sb")

        strip_p = ctx.enter_context(tc.tile_pool(name="strip", bufs=12))
        oT_p = ctx.enter_context(tc.tile_pool(name="oT", bufs=4))
        stage_p = ctx.enter_context(tc.tile_pool(name="stage", bufs=2))
        rec_p = ctx.enter_context(tc.tile_pool(name="rec", bufs=2))
        ostg_p = ctx.enter_context(tc.tile_pool(name="ostg", bufs=6))

        # PSUM: 16KB/partition = 8 banks of 2KB
        st_ps = ctx.enter_context(tc.tile_pool(name="st_ps", bufs=2, space="PSUM"))    # 2x[128,1024] = 4 banks
        a_ps = ctx.enter_context(tc.tile_pool(name="a_ps", bufs=2, space="PSUM"))      # 2x[128,264] = 2 banks
        chainA = ctx.enter_context(tc.tile_pool(name="chA", bufs=1, space="PSUM"))     # [128,512] = 1 bank
        chainB = ctx.enter_context(tc.tile_pool(name="chB", bufs=1, space="PSUM"))     # [128,512] = 1 bank
        chain_pools = [chainA, chainB]
        chain_idx = [0]

        def next_chain_pool():
            p = chain_pools[chain_idx[0] % 2]
            chain_idx[0] += 1
            return p

        fills = _Fills()

        # --- static setup ---
        make_identity(nc, ident[:])
        nc.vector.memset(vv[:, DH::VW], 1.0)       # ones columns
        nc.vector.memset(vv[:, DH + 1::VW], 0.0)   # pad columns

        # PE warmup: the cost model's p-state ramp starts at the PE's first
        # busy stretch and resets on long idles; dummy matmuls spanning the
        # initial DMA wait burn the 3us low-clock window off so the real
        # chains dispatch at full clock.
        warm = next_chain_pool().tile([128, 512], F32, tag="chain",
                                      name="warm_ps")
        warm_in = stage_p.tile([128, 256], BF16, tag="stage", name="warm_in")
        nc.vector.memset(warm_in[:], 0.0)
        for _ in range(WARMUP_MMS):
            nc.tensor.matmul(warm[:, 0:256], ident[:], warm_in[:],
                             start=True, stop=True)
        del warm

        # --- DMA loads: 3D transfers (HWDGE descriptor-gen is ~650ns per
        # DMA instruction, so batch big). wT is split k|q|v and interleaved
        # with the x token-blocks so the k-projection chains start早. ---
        wT3 = wT_d.rearrange("(dc p) e -> p dc e", dc=DC, p=128)
        wTo = wT[:, 0:DC * E3].rearrange("p (dc e) -> p dc e", dc=DC)
        x3i = xT_d.rearrange("(dc p) n -> p dc n", dc=DC, p=128)
        x3o = xT[:, 0:DC * N].rearrange("p (dc n) -> p dc n", dc=DC)

        def x_block(b):
            nc.sync.dma_start(out=x3o[:, :, b * 512:(b + 1) * 512],
                              in_=x3i[:, :, b * 512:(b + 1) * 512])

        nc.sync.dma_start(out=wTo[:, :, 256:512], in_=wT3[:, :, 256:512])  # k
        x_block(0)
        nc.sync.dma_start(out=wTo[:, :, 0:256], in_=wT3[:, :, 0:256])      # q
        x_block(1)
        x_block(2)
        nc.sync.dma_start(out=wTo[:, :, 512:768], in_=wT3[:, :, 512:768])  # v
        x_block(3)
        nc.sync.dma_start(
            out=woS[:, 0:2 * DIM].rearrange("p (d2 e) -> p d2 e", d2=2),
            in_=woT_d.rearrange("(d2 p) e -> p d2 e", d2=2, p=128))
        nc.vector.tensor_copy(_r(woT[:, 0:DIM]), woS[:, 0:DIM])
        nc.vector.tensor_copy(_r(woT[:, DIM:2 * DIM]), woS[:, DIM:2 * DIM])

        # --- qkv projection chains ---
        # ec 0..3 -> qkT block; per (ec, b): accumulate 8 d-chunks
        def _ec_step(ps, ec, b, dc):
            nc.tensor.matmul(
                ps[:],
                wT[:, dc * E3 + ec * 128: dc * E3 + (ec + 1) * 128],
                xT[:, dc * N + b * 512: dc * N + (b + 1) * 512],
                start=(dc == 0), stop=(dc == DC - 1))

        def ec_chain(ec, b):
            ps = next_chain_pool().tile([128, 512], F32, tag="chain",
                                        name="qk_ps")
            for dc in range(DC):
                _ec_step(ps, ec, b, dc)
            nc.vector.tensor_copy(
                qkT[:, ec * N + b * 512: ec * N + (b + 1) * 512], ps[:])

        def ec_atoms(ec, b):
            ref = {}

            def first():
                ref["ps"] = next_chain_pool().tile(
                    [128, 512], F32, tag="chain", name="qk_ps")
                _ec_step(ref["ps"], ec, b, 0)

            atoms = [(EC_STEP_NS, first)]
            for dc in range(1, DC):
                atoms.append((EC_STEP_NS,
                              lambda dc=dc: _ec_step(ref["ps"], ec, b, dc)))
            atoms.append((0.0, lambda: nc.vector.tensor_copy(
                qkT[:, ec * N + b * 512: ec * N + (b + 1) * 512],
                ref["ps"][:])))
            return atoms

        def _v_step(ps, nt, dc):
            nc.tensor.matmul(
                ps[:, 0:256],
                xT[:, dc * N + nt * 128: dc * N + (nt + 1) * 128],
                wT[:, dc * E3 + 512: dc * E3 + 768],
                start=(dc == 0), stop=(dc == DC - 1))

        def _v_copies(ps, nt):
            for h in range(NHL):
                nc.vector.tensor_copy(
                    vv[:, nt * NHL * VW + h * VW: nt * NHL * VW + h * VW + DH],
                    ps[:, h * DH:(h + 1) * DH])

        def v_chain(nt, pool=None):
            if pool is a_ps:
                ps = a_ps.tile([128, NHL * VW], F32, tag="A", name="A_ps")
            else:
                ps = next_chain_pool().tile([128, 512], F32, tag="chain",
                                            name="v_ps")
            for dc in range(DC):
                _v_step(ps, nt, dc)
            _v_copies(ps, nt)

        def v_atoms(nt):
            ref = {}

            def first():
                ref["ps"] = next_chain_pool().tile(
                    [128, 512], F32, tag="chain", name="v_ps")
                _v_step(ref["ps"], nt, 0)

            atoms = [(V_STEP_NS, first)]
            for dc in range(1, DC):
                atoms.append((V_STEP_NS,
                              lambda dc=dc: _v_step(ref["ps"], nt, dc)))
            atoms.append((0.0, lambda: _v_copies(ref["ps"], nt)))
            return atoms

        FUDGE = 1.05  # unmodeled per-instruction overheads
        EC_STEP_NS = _mm_ns(512) * FUDGE
        V_STEP_NS = _mm_ns(256) * FUDGE
        OP_STEP_NS = _mm_ns(512) * FUDGE

        # --- load phase: only the chains gating the very first strips run
        # eagerly; everything else is queued fill work in deadline order.
        # S matmuls only need k per j-chunk, so attention starts as soon as
        # ec2-b0 lands and later k chains are forced per-sg as x blocks
        # stream in; v chains flow into the DMA-stall windows. ---
        ec_chain(2, 0)
        ec_chain(0, 0)
        for nt in range(4):
            v_chain(nt, pool=(a_ps if nt % 2 == 0 else None))

        for b in range(1, NB):
            fills.add_chain(("ec", 2, b), ec_atoms(2, b))
        fills.add_chain(("ec", 0, 1), ec_atoms(0, 1))
        for nt in range(4, 8):
            fills.add_chain(("v", nt), v_atoms(nt))
        fills.add_chain(("ec", 0, 2), ec_atoms(0, 2))
        for nt in range(8, 10):
            fills.add_chain(("v", nt), v_atoms(nt))
        fills.add_chain(("ec", 3, 0), ec_atoms(3, 0))
        for nt in range(10, 12):
            fills.add_chain(("v", nt), v_atoms(nt))
        fills.add_chain(("ec", 0, 3), ec_atoms(0, 3))
        for nt in range(12, 14):
            fills.add_chain(("v", nt), v_atoms(nt))
        fills.add_chain(("ec", 3, 1), ec_atoms(3, 1))
        for nt in range(14, NT):
            fills.add_chain(("v", nt), v_atoms(nt))
        for b in (2, 3):
            fills.add_chain(("ec", 3, b), ec_atoms(3, b))
        for b in range(NB):
            fills.add_chain(("ec", 1, b), ec_atoms(1, b))
        for ec, b in [(2, 0), (0, 0)]:
            fills.done.add(("ec", ec, b))
        for nt in range(4):
            fills.done.add(("v", nt))

        # --- attention machinery ---
        oT_tiles = {}

        def get_oT(it):
            if it not in oT_tiles:
                oT_tiles[it] = oT_p.tile([128, 1024], F32, tag="oT",
                                         name=f"oT{it}")
            return oT_tiles[it]

        def _op_mm(pa, it, ng, eh, d2):
            oT_t = oT_tiles[it]
            nc.tensor.matmul(
                pa,
                _r(oT_t[:, d2 * 512 + ng * 128: d2 * 512 + (ng + 1) * 128]),
                _r(woT[:, d2 * DIM + eh * 512: d2 * DIM + (eh + 1) * 512]),
                start=(d2 == 0), stop=(d2 == 1))

        def _op_store(pa, it, ng, eh, use_dve=True):
            stg = ostg_p.tile([128, 512], F32, tag="ostg")
            nc.vector.tensor_copy(stg[:], pa)
            nc.sync.dma_start(
                out=out_d[it * 512 + ng * 128: it * 512 + (ng + 1) * 128,
                          eh * 512:(eh + 1) * 512],
                in_=stg[:])

        def oproj_one(it, ng, eh, use_st=False):
            if use_st and (ng + eh) % 2 == 0:
                pa = st_ps.tile([128, 1024], F32, tag="st",
                                name="s_ps")[:, 0:512]
            else:
                pa = next_chain_pool().tile([128, 512], F32, tag="chain",
                                            name="op_ps")[:, 0:512]
            for d2 in range(2):
                _op_mm(pa, it, ng, eh, d2)
            _op_store(pa, it, ng, eh, use_dve=(ng + eh) % 2 == 1)

        def op_atoms(it, ng, eh):
            ref = {}

            def first():
                ref["pa"] = next_chain_pool().tile(
                    [128, 512], F32, tag="chain", name="op_ps")[:, 0:512]
                _op_mm(ref["pa"], it, ng, eh, 0)

            return [(OP_STEP_NS, first),
                    (OP_STEP_NS, lambda: _op_mm(ref["pa"], it, ng, eh, 1)),
                    (0.0, lambda: _op_store(ref["pa"], it, ng, eh))]

        # Global deferred-PV queue: entries spill across head loops so a
        # temporary fill backlog never stalls the strip cadence.
        pv = deque()  # (strip_id, h, it, jc, strip, q2, A, epilogue|None)
        strip_id = [0]
        pv_defer = [6]  # first loop: wait for the v0-3 vv copies
        pv_cap = [6]  # max jc pops per strip slot

        def pv_jc(h, jc, strip, q2, A):
            # NOTE: start=True zeroes the whole PSUM bank, so only the very
            # first matmul of the four interleaved ic chains may set it.
            for ic in range(4):
                nc.tensor.matmul(
                    A[:, ic * VW:(ic + 1) * VW],
                    strip[:, q2 * 512 + ic * 128: q2 * 512 + (ic + 1) * 128],
                    vv[:, jc * NHL * VW + h * VW:
                       jc * NHL * VW + h * VW + VW],
                    start=(jc == 0 and ic == 0), stop=(jc == NT - 1))
            fills.pe += 4 * _mm_ns(VW)

        def norm_transp(h, it, A, epilogue_ics=None, enqueue_op=False):
            """Normalize A by its denominator column, transpose to [d, i],
            copy into oT. Per-ic; oproj epilogue closures run after each ic
            when provided (tail pipelining for the final loop)."""
            hb = h // 2
            po = (h % 2) * 64
            rec = rec_p.tile([128, 4], F32, tag="rec")
            with nc.allow_low_precision(reason="denominators are O(100)"):
                nc.vector.reciprocal(rec[:, 0:4], A[:, DH::VW])
            stage = stage_p.tile([128, 256], BF16, tag="stage")
            tp = next_chain_pool().tile([128, 512], F32, tag="chain",
                                        name="tp_ps")
            oT_t = get_oT(it)
            for ic in range(4):
                nc.vector.tensor_scalar_mul(
                    stage[:, ic * DH:(ic + 1) * DH],
                    A[:, ic * VW: ic * VW + DH],
                    rec[:, ic:ic + 1])
                nc.tensor.matmul(
                    tp[po:po + 64, ic * 128:(ic + 1) * 128],
                    stage[:, ic * DH:(ic + 1) * DH],
                    ident[:],
                    start=(ic == 0), stop=(ic == 3))
                if epilogue_ics is None:
                    if ic == 3:
                        nc.vector.tensor_copy(
                            _r(oT_t[po:po + 64, hb * 512:(hb + 1) * 512]),
                            tp[po:po + 64, 0:512])
                else:
                    nc.vector.tensor_copy(
                        _r(oT_t[po:po + 64,
                                hb * 512 + ic * 128: hb * 512 + (ic + 1) * 128]),
                        tp[po:po + 64, ic * 128:(ic + 1) * 128])
                    epilogue_ics(ic)
            fills.pe += 4 * _mm_ns(128)
            fills.done.add(("normed", h, it))
            if enqueue_op:
                for ng in range(4):
                    for eh in range(2):
                        fills.add_chain(("op", it, ng, eh),
                                        op_atoms(it, ng, eh))

        def pump_pv(limit, drain_to=None):
            """Pop deferred PV work. limit = max jc pops (None = all ready);
            drain_to = (h, it): force everything up to that loop's end."""
            n = 0
            while pv:
                sid, h_, it_, jc_, s_, q2_, A_, epi = pv[0]
                if drain_to is None:
                    if sid > strip_id[0] - pv_defer[0]:
                        break
                    if limit is not None and n >= limit:
                        break
                    if ("v", jc_) not in fills.done:
                        break
                else:
                    fills.force(("v", jc_))
                pv.popleft()
                pv_jc(h_, jc_, s_, q2_, A_)
                n += 1
                if epi is not None:
                    epi()
                if drain_to is not None and (h_, it_) == drain_to and jc_ == NT - 1:
                    break

        def head_loop(h, it, tail_epilogue=None):
            """One (head, i-block) attention loop: 8 strips of 2 j-chunks."""
            if os.environ.get("KDBG"):
                print(f"loop h{h} it{it}: pe={fills.pe:.0f} act={fills.act:.0f} "
                      f"pv={len(pv)} q={len(fills.q)}")
            hb = h // 2
            po = (h % 2) * 64
            q_ec = hb            # 0 for h01, 1 for h23
            k_ec = 2 + hb
            fills.force(("ec", q_ec, it))

            A = a_ps.tile([128, NHL * VW], F32, tag="A", name="A_ps")

            for sg in range(8):
                fills.force(("ec", k_ec, sg // 2))
                ps = st_ps.tile([128, 1024], F32, tag="st", name="s_ps")
                for q2 in range(2):
                    jc = sg * 2 + q2
                    nc.tensor.matmul(
                        ps[:, q2 * 512:(q2 + 1) * 512],
                        qkT[po:po + 64,
                            k_ec * N + jc * 128: k_ec * N + (jc + 1) * 128],
                        qkT[po:po + 64,
                            q_ec * N + it * 512: q_ec * N + (it + 1) * 512],
                        start=True, stop=True)
                fills.pe += 2 * _mm_ns(512)
                strip = strip_p.tile([128, 1024], BF16, tag="strip")
                nc.scalar.activation(strip[:], ps[:], EXP, scale=SCALE)
                fills.act += ACT_STRIP_NS
                strip_id[0] += 1

                for q2 in range(2):
                    jc = sg * 2 + q2
                    epi = None
                    if jc == NT - 1:
                        eop = (h == 3 and it < NB - 1)
                        epi = (lambda h=h, it=it, A=A, te=tail_epilogue,
                               eop=eop: norm_transp(h, it, A, te, eop))
                    pv.append((strip_id[0], h, it, jc, strip, q2, A, epi))
                pump_pv(pv_cap[0])
                fills.pump()

        # --- attention: h0/h1 over all i-blocks, then (i-block, h2, h3) ---
        for h in (0, 1):
            for it in range(NB):
                head_loop(h, it)
                pv_defer[0] = 2
        for it in range(NB):
            for h in (2, 3):
                last = (it == NB - 1 and h == 3)
                tail = None
                if last:
                    def tail(ic, it=it):
                        for eh in range(2):
                            oproj_one(it, ic, eh, use_st=True)
                head_loop(h, it, tail_epilogue=tail)
        pump_pv(None, drain_to=(3, NB - 1))
        fills.drain()
    if split_waits:
        _split_dma_waits(nc)
    return nc


def _split_dma_waits(nc):
    """walrus's DMA/LDWEIGHTS encodings take a single sync wait; move
    extra waits onto an EventSemaphore on the issuing sequencer."""
    fn = nc.m.functions[0]
    for bb in fn.blocks:
        insts = bb.instructions
        i = 0
        while i < len(insts):
            inst = insts[i]
            si = getattr(inst, "sync_info", None)
            if (si is not None and len(si.on_wait) > 1
                    and type(inst).__name__ != "InstEventSemaphore"):
                waits = list(si.on_wait)
                for k, w in enumerate(waits[:-1]):
                    ev = mybir.InstEventSemaphore(
                        name=f"{inst.name}-wsplit{k}", ins=[], outs=[])
                    ev.engine = inst.engine
                    ev.sync_info = type(si)(on_wait=[w], on_update=[])
                    insts.insert(i, ev)
                    i += 1
                inst.sync_info = type(si)(on_wait=waits[-1:],
                                          on_update=list(si.on_update))
            i += 1


_NC = None


def _get_nc():
    global _NC
    if _NC is None:
        _NC = build_nc()
    return _NC


def make_in_maps(x, w_qkv, w_out):
    import ml_dtypes
    bf = ml_dtypes.bfloat16
    x = np.asarray(x, dtype=np.float32)
    w_qkv = np.asarray(w_qkv, dtype=np.float32)
    w_out = np.asarray(w_out, dtype=np.float32)
    xT_by_b = [np.ascontiguousarray(x[b].T).astype(bf) for b in range(B)]
    in_maps = []
    for c in range(NCORES):
        b, g = divmod(c, 4)
        r0 = g * NHL * DH  # 256-wide feature slice
        wq = w_qkv[r0:r0 + EV]
        wk = w_qkv[DIM + r0:DIM + r0 + EV]
        wv = w_qkv[2 * DIM + r0:2 * DIM + r0 + EV]
        wT = np.ascontiguousarray(np.concatenate([wq, wk, wv], 0).T).astype(bf)
        woT = np.ascontiguousarray(w_out[:, r0:r0 + EV].T)
        in_maps.append({"xT": xT_by_b[b], "wT": wT, "woT": woT})
    return in_maps


def combine(results, b_out):
    """results: list of 8 dicts with 'out' [N, DIM] partials."""
    b_out = np.asarray(b_out, dtype=np.float32)
    out = np.empty((B, N, DIM), dtype=np.float32)
    for b in range(B):
        acc = results[4 * b]["out"].astype(np.float32, copy=True)
        for g in range(1, 4):
            acc += results[4 * b + g]["out"]
        out[b] = acc + b_out[None, :]
    return out


def kernel(x, w_qkv, w_out, b_out):
    from concourse.bass_utils import run_bass_kernel_spmd
    nc = _get_nc()
    in_maps = make_in_maps(x, w_qkv, w_out)
    res = run_bass_kernel_spmd(nc, in_maps, list(range(NCORES)))
    return combine(res.results, b_out)
